# revision 3
# baseline (speedup 1.0000x reference)
"""Trainium2 Bass kernel for nn_BloqueAttn: causal RoPE attention, 16 heads,
head-sharded (tensor-parallel) across 8 NeuronCores, o_proj row-sharded with
host-side all-reduce of the partials.

Self-contained: hardcodes shapes B=1, L=4096, D=1024, H=16, DH=64, 8 cores.
"""
import math

import numpy as np
import ml_dtypes

import concourse.bass as bass
import concourse.mybir as mybir
import concourse.tile as tile
from concourse.bass_utils import run_bass_kernel_spmd

F32 = mybir.dt.float32
F32R = mybir.dt.float32r
BF16 = mybir.dt.bfloat16

B, L, D = 1, 4096, 1024
H, DH = 16, 64
BASE = 10000.0
N_CORES = 8
HPC = H // N_CORES          # heads per core = 2
DH2 = HPC * DH              # packed head dim = 128
SCALE = DH ** -0.5          # 0.125

NEG = -1e30


# ---------------------------------------------------------------- host helpers

def _rope_tables(L_, dh):
    inv_freq = 1.0 / (BASE ** (np.arange(0, dh, 2, dtype=np.float32) / dh))
    freqs = np.outer(np.arange(L_, dtype=np.float32), inv_freq)  # [L, 32]
    return np.cos(freqs).astype(np.float32), np.sin(freqs).astype(np.float32)


def _host_consts(L_):
    cos, sin = _rope_tables(L_, DH)          # [L, 32]
    cosT, sinT = cos.T.copy(), sin.T.copy()  # [32, L]
    cos_stack = np.concatenate([cosT, cosT, cosT, cosT], 0)          # [128, L]
    sin_signed = np.concatenate([-sinT, sinT, -sinT, sinT], 0)       # [128, L]

    j = np.arange(128)[:, None]
    c = np.arange(128)[None, :]
    trilneg = np.where(j > c, NEG, 0.0).astype(np.float32)           # [128,128]
    r3mask = np.concatenate(
        [np.full((128, 128), NEG, np.float32), trilneg], axis=1)     # [128,256]

    ident = np.eye(128, dtype=np.float32)
    sel2 = np.zeros((2, 128), np.float32)
    sel2[0, 0:64] = 1.0
    sel2[1, 64:128] = 1.0
    ones128 = np.ones((128, 1), np.float32)

    return {
        "cos_st": cos_stack,
        "sin_st": sin_signed,
        "trilneg": trilneg.astype(ml_dtypes.bfloat16),
        "r3mask": r3mask.astype(ml_dtypes.bfloat16),
        "ident_b": ident.astype(ml_dtypes.bfloat16),
        "ident_f": ident,
        "sel2": sel2,
        "ones": ones128,
    }


def _core_weights(core, Wq, Wk, Wv, Wo):
    """Per-core transposed weight slices with RoPE even/odd permutation."""
    perm = np.concatenate([np.arange(0, DH, 2), np.arange(1, DH, 2)])  # [64]
    rows_p, rows = [], []
    for hh in (HPC * core, HPC * core + 1):
        rows_p.append(DH * hh + perm)
        rows.append(DH * hh + np.arange(DH))
    rows_p = np.concatenate(rows_p)
    rows = np.concatenate(rows)
    wqT = np.ascontiguousarray(Wq[rows_p, :].T)   # [D, 128]
    wkT = np.ascontiguousarray(Wk[rows_p, :].T)   # [D, 128]
    wvT = np.ascontiguousarray(Wv[rows, :].T)     # [D, 128]
    woC = np.ascontiguousarray(Wo[:, DH2 * core: DH2 * (core + 1)].T)  # [128, D]
    return wqT, wkT, wvT, woC


# ---------------------------------------------------------------- device emit

def emit(nc, tc, aps, L_):
    """Emit the per-core program. aps: dict of dram APs."""
    NLT = L_ // 512           # number of 512-wide L tiles
    NKB = L_ // 128           # number of 128-wide k blocks
    NLC = L_ // 128           # number of 128-wide L chunks (o_proj)
    ND = D // 128             # D chunks = 8
    NQB = NLT                 # q tiles of 512

    xt, wq, wk, wv, wo = aps["xt"], aps["wq"], aps["wk"], aps["wv"], aps["wo"]
    partial = aps["partial"]

    with tc.tile_pool(name="persist", bufs=1) as pp:
        # persistent SBUF tensors
        wq_sb = pp.tile([128, D], F32)
        wk_sb = pp.tile([128, D], F32)
        wv_sb = pp.tile([128, D], F32)
        wo_sb = pp.tile([128, D], F32)
        cos_sb = pp.tile([128, L_], F32)
        sin_sb = pp.tile([128, L_], F32)
        tril_sb = pp.tile([128, 128], BF16)
        r3m_sb = pp.tile([128, 256], BF16)
        idb_sb = pp.tile([128, 128], BF16)
        idf_sb = pp.tile([128, 128], F32)
        sel2_sb = pp.tile([2, 128], F32)
        ones_sb = pp.tile([128, 1], F32)
        qT = pp.tile([128, L_], F32)
        kT = pp.tile([128, L_], F32)
        v_nat = pp.tile([128, L_], F32)
        OT = pp.tile([128, L_], F32)
        sums_keep = pp.tile([33, L_], F32)
        stack16 = pp.tile([2 * NQB, 512], F32)
        inv16 = pp.tile([2 * NQB, 512], F32)
        inv2 = pp.tile([2, L_], F32)

        # weight loads: dram [D, 128] -> sbuf [128, ND*128] chunkwise
        for name, dst in (("wq", wq_sb), ("wk", wk_sb), ("wv", wv_sb)):
            for ch in range(ND):
                nc.sync.dma_start(dst[:, bass.ts(ch, 128)],
                                  aps[name][bass.ts(ch, 128), :])
        nc.sync.dma_start(wo_sb[:], wo[:])
        nc.sync.dma_start(cos_sb[:], aps["cos_st"][:])
        nc.sync.dma_start(sin_sb[:], aps["sin_st"][:])
        nc.sync.dma_start(tril_sb[:], aps["trilneg"][:])
        nc.sync.dma_start(r3m_sb[:], aps["r3mask"][:])
        nc.sync.dma_start(idb_sb[:], aps["ident_b"][:])
        nc.sync.dma_start(idf_sb[:], aps["ident_f"][:])
        nc.sync.dma_start(sel2_sb[:], aps["sel2"][:])
        nc.sync.dma_start(ones_sb[:], aps["ones"][:])

        # ---------------- Phase A: projections + RoPE + V transpose ----------
        with tc.tile_pool(name="psA", bufs=2, space="PSUM") as psA, \
             tc.tile_pool(name="sbA", bufs=3) as sbA:
            for lt in range(NLT):
                sl = bass.ts(lt, 512)
                qps = psA.tile([128, 512], F32, tag="qps")
                kps = psA.tile([128, 512], F32, tag="kps")
                vps = psA.tile([128, 512], F32, tag="vps")
                for ch in range(ND):
                    xt_t = sbA.tile([128, 512], F32, tag="xt", bufs=4)
                    nc.sync.dma_start(xt_t[:], xt[bass.ts(ch, 128), sl])
                    st, sp = ch == 0, ch == ND - 1
                    xr = xt_t[:].bitcast(F32R)
                    nc.tensor.matmul(qps[:], wq_sb[:, bass.ts(ch, 128)].bitcast(F32R), xr, start=st, stop=sp)
                    nc.tensor.matmul(kps[:], wk_sb[:, bass.ts(ch, 128)].bitcast(F32R), xr, start=st, stop=sp)
                    nc.tensor.matmul(vps[:], wv_sb[:, bass.ts(ch, 128)].bitcast(F32R), xr, start=st, stop=sp)
                # RoPE for q and k
                for ps, dst in ((qps, qT), (kps, kT)):
                    raw = sbA.tile([128, 512], F32, tag="raw")
                    swp = sbA.tile([128, 512], F32, tag="swp")
                    tmp = sbA.tile([128, 512], F32, tag="tmp")
                    nc.vector.tensor_copy(raw[:], ps[:])
                    for b0 in range(4):
                        src_b = (b0 ^ 1) * 32
                        nc.sync.dma_start(swp[b0 * 32:(b0 + 1) * 32, :],
                                          raw[src_b:src_b + 32, :])
                    nc.vector.tensor_mul(dst[:, sl], raw[:], cos_sb[:, sl])
                    nc.vector.tensor_mul(tmp[:], swp[:], sin_sb[:, sl])
                    nc.vector.tensor_add(dst[:, sl], dst[:, sl], tmp[:])
                # V: transpose [dh2, L] tile -> natural [kpos, dh2]
                vt = sbA.tile([128, 512], F32, tag="vt")
                nc.vector.tensor_copy(vt[:], vps[:])
                for j in range(4):
                    kb = 4 * lt + j
                    trp = psA.tile([128, 128], F32, tag="trp")
                    nc.tensor.transpose(trp[:], vt[:, bass.ts(j, 128)], idf_sb[:])
                    nc.vector.tensor_copy(v_nat[:, bass.ts(kb, 128)], trp[:])

        # ---------------- Phase B: attention ---------------------------------
        with tc.tile_pool(name="psB", bufs=1, space="PSUM") as psB, \
             tc.tile_pool(name="sbB", bufs=6) as sbB:
            for qb in range(NQB):
                qsl0 = 512 * qb
                ov0 = psB.tile([128, 512], F32, tag="pv0")
                ov1 = psB.tile([128, 512], F32, tag="pv1")
                sm0 = psB.tile([128, 512], F32, tag="sm0")
                sm1 = psB.tile([128, 512], F32, tag="sm1")
                nkb = 4 * qb + 4
                for kb in range(nkb):
                    r = kb - 4 * qb
                    if r < 0:
                        c0 = 0
                    elif r < 3:
                        c0 = 128 * r
                    else:
                        c0 = 256
                    W = 512 - c0
                    ksl = bass.ts(kb, 128)
                    qsl = bass.ds(qsl0 + c0, W)
                    s0 = psB.tile([128, 512], F32, tag="sc0", bufs=2)
                    s1 = psB.tile([128, 512], F32, tag="sc1", bufs=2)
                    diag = r >= 0
                    nc.tensor.matmul(s0[:, c0:512], kT[0:64, ksl].bitcast(F32R),
                                     qT[0:64, qsl].bitcast(F32R),
                                     start=True, stop=not diag)
                    nc.tensor.matmul(s1[:, c0:512], kT[64:128, ksl].bitcast(F32R),
                                     qT[64:128, qsl].bitcast(F32R),
                                     start=True, stop=not diag)
                    if diag:
                        if r < 3:
                            msl = slice(128 * r, 128 * r + 128)
                            mk = tril_sb
                        else:
                            msl = slice(256, 512)
                            mk = r3m_sb
                        nc.tensor.matmul(s0[:, msl], idb_sb[:], mk[:], start=False, stop=True)
                        nc.tensor.matmul(s1[:, msl], idb_sb[:], mk[:], start=False, stop=True)
                    p0 = sbB.tile([128, 512], F32, tag="p0")
                    p1 = sbB.tile([128, 512], F32, tag="p1")
                    nc.scalar.activation(p0[:, c0:512], s0[:, c0:512],
                                         mybir.ActivationFunctionType.Exp, scale=SCALE)
                    nc.scalar.activation(p1[:, c0:512], s1[:, c0:512],
                                         mybir.ActivationFunctionType.Exp, scale=SCALE)
                    st, sp = kb == 0, kb == nkb - 1
                    p0r = p0[:, c0:512].bitcast(F32R)
                    p1r = p1[:, c0:512].bitcast(F32R)
                    vsl0 = bass.ds(128 * kb, 64)
                    vsl1 = bass.ds(128 * kb + 64, 64)
                    nc.tensor.matmul(ov0[0:64, c0:512], v_nat[:, vsl0].bitcast(F32R), p0r, start=st, stop=sp)
                    nc.tensor.matmul(ov1[64:128, c0:512], v_nat[:, vsl1].bitcast(F32R), p1r, start=st, stop=sp)
                    nc.tensor.matmul(sm0[0:1, c0:512], ones_sb[:].bitcast(F32R), p0r, start=st, stop=sp)
                    nc.tensor.matmul(sm1[32:33, c0:512], ones_sb[:].bitcast(F32R), p1r, start=st, stop=sp)
                osl = bass.ds(qsl0, 512)
                nc.vector.tensor_copy(OT[0:64, osl], ov0[0:64, :])
                nc.vector.tensor_copy(OT[64:128, osl], ov1[64:128, :])
                nc.vector.tensor_copy(sums_keep[0:1, osl], sm0[0:1, :])
                nc.vector.tensor_copy(sums_keep[32:33, osl], sm1[32:33, :])

        # ---------------- normalization --------------------------------------
        with tc.tile_pool(name="psN", bufs=2, space="PSUM") as psN:
            for qb in range(NQB):
                osl = bass.ts(qb, 512)
                nc.sync.dma_start(stack16[qb:qb + 1, :], sums_keep[0:1, osl])
                nc.sync.dma_start(stack16[NQB + qb:NQB + qb + 1, :],
                                  sums_keep[32:33, osl])
            nc.vector.reciprocal(inv16[:], stack16[:])
            for qb in range(NQB):
                osl = bass.ts(qb, 512)
                nc.sync.dma_start(inv2[0:1, osl], inv16[qb:qb + 1, :])
                nc.sync.dma_start(inv2[1:2, osl], inv16[NQB + qb:NQB + qb + 1, :])
            for qb in range(NQB):
                osl = bass.ts(qb, 512)
                bc = psN.tile([128, 512], F32, tag="bc")
                nc.tensor.matmul(bc[:], sel2_sb[:].bitcast(F32R),
                                 inv2[:, osl].bitcast(F32R), start=True, stop=True)
                nc.vector.tensor_mul(OT[:, osl], OT[:, osl], bc[:])

        # ---------------- Phase C: o_proj ------------------------------------
        with tc.tile_pool(name="psC", bufs=4, space="PSUM") as psC, \
             tc.tile_pool(name="sbC", bufs=4) as sbC:
            for lc in range(NLC):
                for n in range(D // 512):
                    op = psC.tile([128, 512], F32, tag="op")
                    nc.tensor.matmul(op[:], OT[:, bass.ts(lc, 128)].bitcast(F32R),
                                     wo_sb[:, bass.ts(n, 512)].bitcast(F32R),
                                     start=True, stop=True)
                    ob = sbC.tile([128, 512], F32, tag="ob")
                    nc.vector.tensor_copy(ob[:], op[:])
                    nc.sync.dma_start(partial[bass.ts(lc, 128), bass.ts(n, 512)], ob[:])


def build(L_=L, debug=False):
    nc = bass.Bass("TRN2", target_bir_lowering=False, debug=debug,
                   enable_asserts=False)
    aps = {}
    aps["xt"] = nc.dram_tensor("xt", [D, L_], F32, kind="ExternalInput").ap()
    aps["wq"] = nc.dram_tensor("wq", [D, 128], F32, kind="ExternalInput").ap()
    aps["wk"] = nc.dram_tensor("wk", [D, 128], F32, kind="ExternalInput").ap()
    aps["wv"] = nc.dram_tensor("wv", [D, 128], F32, kind="ExternalInput").ap()
    aps["wo"] = nc.dram_tensor("wo", [128, D], F32, kind="ExternalInput").ap()
    aps["cos_st"] = nc.dram_tensor("cos_st", [128, L_], F32, kind="ExternalInput").ap()
    aps["sin_st"] = nc.dram_tensor("sin_st", [128, L_], F32, kind="ExternalInput").ap()
    aps["trilneg"] = nc.dram_tensor("trilneg", [128, 128], BF16, kind="ExternalInput").ap()
    aps["r3mask"] = nc.dram_tensor("r3mask", [128, 256], BF16, kind="ExternalInput").ap()
    aps["ident_b"] = nc.dram_tensor("ident_b", [128, 128], BF16, kind="ExternalInput").ap()
    aps["ident_f"] = nc.dram_tensor("ident_f", [128, 128], F32, kind="ExternalInput").ap()
    aps["sel2"] = nc.dram_tensor("sel2", [2, 128], F32, kind="ExternalInput").ap()
    aps["ones"] = nc.dram_tensor("ones", [128, 1], F32, kind="ExternalInput").ap()
    aps["partial"] = nc.dram_tensor("partial", [L_, D], F32, kind="ExternalOutput").ap()

    with tile.TileContext(nc) as tc:
        emit(nc, tc, aps, L_)
    return nc, aps


def make_in_maps(x, Wq, Wk, Wv, Wo, L_=L):
    xT = np.ascontiguousarray(x.reshape(L_, D).T).astype(np.float32)
    consts = _host_consts(L_)
    in_maps = []
    for c in range(N_CORES):
        wqT, wkT, wvT, woC = _core_weights(c, Wq, Wk, Wv, Wo)
        m = {"xt": xT, "wq": wqT, "wk": wkT, "wv": wvT, "wo": woC}
        m.update(consts)
        in_maps.append(m)
    return in_maps


_CACHE = {}


def _run(inputs, trace=False, **kw):
    x = np.asarray(inputs["x"], np.float32)
    Wq = np.asarray(inputs["Wq"], np.float32)
    Wk = np.asarray(inputs["Wk"], np.float32)
    Wv = np.asarray(inputs["Wv"], np.float32)
    Wo = np.asarray(inputs["Wo"], np.float32)
    if "nc" not in _CACHE:
        _CACHE["nc"] = build()[0]
    nc = _CACHE["nc"]
    in_maps = make_in_maps(x, Wq, Wk, Wv, Wo)
    res = run_bass_kernel_spmd(nc, in_maps, core_ids=list(range(N_CORES)),
                               trace=trace, **kw)
    acc = np.zeros((L, D), np.float64)
    for r in res.results:
        acc += r["partial"].astype(np.float64)
    out = acc.astype(np.float32).reshape(B, L, D)
    return out, res


def kernel(**inputs):
    out, _ = _run(inputs)
    return out


# revision 11
# speedup vs baseline: 1.0319x; 1.0319x over previous
"""Trainium2 Bass kernel for nn_BloqueAttn: causal RoPE attention, 16 heads,
head-sharded (tensor-parallel) across 8 NeuronCores, o_proj row-sharded with
host-side all-reduce of the partials.

Self-contained: hardcodes shapes B=1, L=4096, D=1024, H=16, DH=64, 8 cores.
"""
import math

import numpy as np
import ml_dtypes

import concourse.bass as bass
import concourse.bacc as bacc
import concourse.mybir as mybir
import concourse.tile as tile
from concourse.bass_utils import run_bass_kernel_spmd

F32 = mybir.dt.float32
F32R = mybir.dt.float32r
BF16 = mybir.dt.bfloat16

B, L, D = 1, 4096, 1024
H, DH = 16, 64
BASE = 10000.0
N_CORES = 8
HPC = H // N_CORES          # heads per core = 2
DH2 = HPC * DH              # packed head dim = 128
SCALE = DH ** -0.5          # 0.125

NEG = -1e30


# ---------------------------------------------------------------- host helpers

def _rope_tables(L_, dh):
    inv_freq = 1.0 / (BASE ** (np.arange(0, dh, 2, dtype=np.float32) / dh))
    freqs = np.outer(np.arange(L_, dtype=np.float32), inv_freq)  # [L, 32]
    return np.cos(freqs).astype(np.float32), np.sin(freqs).astype(np.float32)


def _host_consts(L_):
    cos, sin = _rope_tables(L_, DH)          # [L, 32]
    cosT, sinT = cos.T.copy(), sin.T.copy()  # [32, L]
    cos_stack = np.concatenate([cosT, cosT, cosT, cosT], 0)          # [128, L]
    sin_signed = np.concatenate([-sinT, sinT, -sinT, sinT], 0)       # [128, L]

    j = np.arange(128)[:, None]
    c = np.arange(128)[None, :]
    trilneg = np.where(j > c, NEG, 0.0).astype(np.float32)           # [128,128]
    r3mask = np.concatenate(
        [np.full((128, 128), NEG, np.float32), trilneg], axis=1)     # [128,256]

    ident = np.eye(128, dtype=np.float32)
    sel2 = np.zeros((2, 128), np.float32)
    sel2[0, 0:64] = 1.0
    sel2[1, 64:128] = 1.0
    return {
        "cos_st": cos_stack,
        "sin_st": sin_signed,
        "trilneg": trilneg.astype(ml_dtypes.bfloat16),
        "r3mask": r3mask.astype(ml_dtypes.bfloat16),
        "ident_b": ident.astype(ml_dtypes.bfloat16),
        "ident_f": ident,
        "sel2": sel2,
    }


def _core_weights(core, Wq, Wk, Wv, Wo):
    """Per-core transposed weight slices with RoPE even/odd permutation."""
    perm = np.concatenate([np.arange(0, DH, 2), np.arange(1, DH, 2)])  # [64]
    rows_p, rows = [], []
    for hh in (HPC * core, HPC * core + 1):
        rows_p.append(DH * hh + perm)
        rows.append(DH * hh + np.arange(DH))
    rows_p = np.concatenate(rows_p)
    rows = np.concatenate(rows)
    wqT = np.ascontiguousarray(Wq[rows_p, :].T)   # [D, 128]
    wkT = np.ascontiguousarray(Wk[rows_p, :].T)   # [D, 128]
    wvT = np.ascontiguousarray(Wv[rows, :].T)     # [D, 128]
    woC = np.ascontiguousarray(Wo[:, DH2 * core: DH2 * (core + 1)].T)  # [128, D]
    return wqT, wkT, wvT, woC


# ---------------------------------------------------------------- device emit

def emit(nc, tc, aps, L_):
    """Emit the per-core program. aps: dict of dram APs."""
    NLT = L_ // 512           # number of 512-wide L tiles
    NKB = L_ // 128           # number of 128-wide k blocks
    NLC = L_ // 128           # number of 128-wide L chunks (o_proj)
    ND = D // 128             # D chunks = 8
    NQB = NLT                 # q tiles of 512

    xt, wq, wk, wv, wo = aps["xt"], aps["wq"], aps["wk"], aps["wv"], aps["wo"]
    partial = aps["partial"]

    with tc.tile_pool(name="persist", bufs=1) as pp:
        # persistent SBUF tensors
        wq_c = [pp.tile([128, 128], F32R, name=f"wq_c{i}") for i in range(ND)]
        wk_c = [pp.tile([128, 128], F32R, name=f"wk_c{i}") for i in range(ND)]
        wv_c = [pp.tile([128, 128], F32R, name=f"wv_c{i}") for i in range(ND)]
        wo_sb = pp.tile([128, D], F32R)
        cos_sb = pp.tile([128, L_], F32)
        sin_sb = pp.tile([128, L_], F32)
        tril_sb = pp.tile([128, 128], BF16)
        r3m_sb = pp.tile([128, 256], BF16)
        idb_sb = pp.tile([128, 128], BF16)
        idf_sb = pp.tile([128, 128], F32)
        sel2_sb = pp.tile([2, 128], F32)
        qT = pp.tile([128, L_], F32R)
        kT = pp.tile([128, L_], F32R)
        v_nat = pp.tile([128, (L_ // 128) * 130], F32R)
        OT = pp.tile([128, L_], F32R)
        sums_keep = pp.tile([65, 2 * L_], F32)
        stack16 = pp.tile([2 * NQB, 512], F32)
        inv16 = pp.tile([2 * NQB, 512], F32)
        inv2 = pp.tile([2, L_], F32)

        # weight loads: dram [D, 128] -> per-chunk sbuf tiles [128, 128]
        for name, dsts in (("wq", wq_c), ("wk", wk_c), ("wv", wv_c)):
            for ch in range(ND):
                nc.sync.dma_start(dsts[ch][:], aps[name][bass.ts(ch, 128), :])
        nc.sync.dma_start(wo_sb[:], wo[:])
        nc.sync.dma_start(cos_sb[:], aps["cos_st"][:])
        nc.sync.dma_start(sin_sb[:], aps["sin_st"][:])
        nc.sync.dma_start(tril_sb[:], aps["trilneg"][:])
        nc.sync.dma_start(r3m_sb[:], aps["r3mask"][:])
        nc.sync.dma_start(idb_sb[:], aps["ident_b"][:])
        nc.sync.dma_start(idf_sb[:], aps["ident_f"][:])
        nc.sync.dma_start(sel2_sb[:], aps["sel2"][:])

        nc.gpsimd.memset(v_nat[:].bitcast(F32), 1.0)

        # ---------------- Phase A: projections + RoPE + V transpose ----------
        with tc.tile_pool(name="psA", bufs=2, space="PSUM") as psA, \
             tc.tile_pool(name="sbA", bufs=3) as sbA:
            for lt in range(NLT):
                sl = bass.ts(lt, 512)
                qps = psA.tile([128, 512], F32, tag="qps")
                kps = psA.tile([128, 512], F32, tag="kps")
                vps = psA.tile([128, 512], F32, tag="vps")
                for ch in range(ND):
                    xt_t = sbA.tile([128, 512], F32R, tag="xt", bufs=4)
                    nc.sync.dma_start(xt_t[:], xt[bass.ts(ch, 128), sl])
                    st, sp = ch == 0, ch == ND - 1
                    xr = xt_t[:]
                    nc.tensor.matmul(qps[:], wq_c[ch][:], xr, start=st, stop=sp)
                    nc.tensor.matmul(kps[:], wk_c[ch][:], xr, start=st, stop=sp)
                    nc.tensor.matmul(vps[:], wv_c[ch][:], xr, start=st, stop=sp)
                # RoPE for q and k
                for ps, dst in ((qps, qT), (kps, kT)):
                    raw = sbA.tile([128, 512], F32, tag="raw")
                    swp = sbA.tile([128, 512], F32, tag="swp")
                    tmp = sbA.tile([128, 512], F32, tag="tmp")
                    nc.vector.tensor_copy(raw[:], ps[:])
                    for b0 in range(4):
                        src_b = (b0 ^ 1) * 32
                        nc.sync.dma_start(swp[b0 * 32:(b0 + 1) * 32, :],
                                          raw[src_b:src_b + 32, :])
                    nc.vector.tensor_mul(dst[:, sl], raw[:], cos_sb[:, sl])
                    nc.vector.tensor_mul(tmp[:], swp[:], sin_sb[:, sl])
                    nc.vector.tensor_add(dst[:, sl], dst[:, sl], tmp[:])
                # V: transpose [dh2, L] tile -> natural [kpos, dh2]
                vt = sbA.tile([128, 512], F32, tag="vt")
                nc.vector.tensor_copy(vt[:], vps[:])
                for j in range(4):
                    kb = 4 * lt + j
                    trp = psA.tile([128, 128], F32, tag="trp")
                    nc.tensor.transpose(trp[:], vt[:, bass.ts(j, 128)], idf_sb[:])
                    nc.vector.tensor_copy(v_nat[:, 130 * kb:130 * kb + 64], trp[:, 0:64])
                    nc.vector.tensor_copy(v_nat[:, 130 * kb + 65:130 * kb + 129], trp[:, 64:128])

        # ---------------- Phase B: attention ---------------------------------
        with tc.tile_pool(name="psB", bufs=1, space="PSUM") as psB, \
             tc.tile_pool(name="sbB", bufs=6) as sbB:
            for qb in range(NQB):
                qsl0 = 512 * qb
                ov0 = psB.tile([128, 512], F32, tag="pv0", bufs=2)
                ov1 = psB.tile([128, 512], F32, tag="pv1", bufs=2)
                nkb = 4 * qb + 4
                for kb in range(nkb):
                    r = kb - 4 * qb
                    if r < 0:
                        c0 = 0
                    elif r < 3:
                        c0 = 128 * r
                    else:
                        c0 = 256
                    W = 512 - c0
                    ksl = bass.ts(kb, 128)
                    qsl = bass.ds(qsl0 + c0, W)
                    s0 = psB.tile([128, 512], F32, tag="sc0", bufs=2)
                    s1 = psB.tile([128, 512], F32, tag="sc1", bufs=2)
                    diag = r >= 0
                    nc.tensor.matmul(s0[:, c0:512], kT[0:64, ksl],
                                     qT[0:64, qsl],
                                     start=True, stop=not diag)
                    nc.tensor.matmul(s1[:, c0:512], kT[64:128, ksl],
                                     qT[64:128, qsl],
                                     start=True, stop=not diag)
                    if diag:
                        if r < 3:
                            msl = slice(128 * r, 128 * r + 128)
                            mk = tril_sb
                        else:
                            msl = slice(256, 512)
                            mk = r3m_sb
                        nc.tensor.matmul(s0[:, msl], idb_sb[:], mk[:], start=False, stop=True)
                        nc.tensor.matmul(s1[:, msl], idb_sb[:], mk[:], start=False, stop=True)
                    p0 = sbB.tile([128, 512], F32R, tag="p0")
                    p1 = sbB.tile([128, 512], F32R, tag="p1")
                    nc.scalar.activation(p0[:, c0:512], s0[:, c0:512],
                                         mybir.ActivationFunctionType.Exp, scale=SCALE)
                    nc.scalar.activation(p1[:, c0:512], s1[:, c0:512],
                                         mybir.ActivationFunctionType.Exp, scale=SCALE)
                    st, sp = kb == 0, kb == nkb - 1
                    p0r = p0[:, c0:512]
                    p1r = p1[:, c0:512]
                    vsl0 = bass.ds(130 * kb, 65)
                    vsl1 = bass.ds(130 * kb + 65, 65)
                    nc.tensor.matmul(ov0[0:65, c0:512], v_nat[:, vsl0], p0r, start=st, stop=sp)
                    nc.tensor.matmul(ov1[0:65, c0:512], v_nat[:, vsl1], p1r, start=st, stop=sp)
                osl = bass.ds(qsl0, 512)
                nc.vector.tensor_copy(OT[0:64, osl], ov0[0:64, :])
                o1t = sbB.tile([64, 512], F32R, tag="o1t", bufs=2)
                nc.vector.tensor_copy(o1t[:], ov1[0:64, :])
                nc.sync.dma_start(OT[64:128, osl], o1t[:])
                nc.vector.tensor_copy(sums_keep[64:65, osl], ov0[64:65, :])
                nc.vector.tensor_copy(sums_keep[64:65, bass.ds(L_ + qsl0, 512)],
                                      ov1[64:65, :])

        # ---------------- normalization --------------------------------------
        with tc.tile_pool(name="psN", bufs=2, space="PSUM") as psN:
            for qb in range(NQB):
                osl = bass.ts(qb, 512)
                nc.sync.dma_start(stack16[qb:qb + 1, :], sums_keep[64:65, osl])
                nc.sync.dma_start(stack16[NQB + qb:NQB + qb + 1, :],
                                  sums_keep[64:65, bass.ds(L_ + 512 * qb, 512)])
            nc.vector.reciprocal(inv16[:], stack16[:])
            for qb in range(NQB):
                osl = bass.ts(qb, 512)
                nc.sync.dma_start(inv2[0:1, osl], inv16[qb:qb + 1, :])
                nc.sync.dma_start(inv2[1:2, osl], inv16[NQB + qb:NQB + qb + 1, :])
            for qb in range(NQB):
                osl = bass.ts(qb, 512)
                bc = psN.tile([128, 512], F32, tag="bc")
                nc.tensor.matmul(bc[:], sel2_sb[:],
                                 inv2[:, osl], start=True, stop=True)
                nc.vector.tensor_mul(OT[:, osl], OT[:, osl], bc[:])

        # ---------------- Phase C: o_proj ------------------------------------
        with tc.tile_pool(name="psC", bufs=4, space="PSUM") as psC, \
             tc.tile_pool(name="sbC", bufs=4) as sbC:
            for lc in range(NLC):
                for n in range(D // 512):
                    op = psC.tile([128, 512], F32, tag="op")
                    nc.tensor.matmul(op[:], OT[:, bass.ts(lc, 128)],
                                     wo_sb[:, bass.ts(n, 512)],
                                     start=True, stop=True)
                    ob = sbC.tile([128, 512], F32, tag="ob")
                    nc.vector.tensor_copy(ob[:], op[:])
                    nc.sync.dma_start(partial[bass.ts(lc, 128), bass.ts(n, 512)], ob[:])


def build(L_=L, debug=False):
    nc = bacc.Bacc("TRN2", target_bir_lowering=False, debug=debug,
                   enable_asserts=False)
    aps = {}
    aps["xt"] = nc.dram_tensor("xt", [D, L_], F32R, kind="ExternalInput").ap()
    aps["wq"] = nc.dram_tensor("wq", [D, 128], F32R, kind="ExternalInput").ap()
    aps["wk"] = nc.dram_tensor("wk", [D, 128], F32R, kind="ExternalInput").ap()
    aps["wv"] = nc.dram_tensor("wv", [D, 128], F32R, kind="ExternalInput").ap()
    aps["wo"] = nc.dram_tensor("wo", [128, D], F32R, kind="ExternalInput").ap()
    aps["cos_st"] = nc.dram_tensor("cos_st", [128, L_], F32, kind="ExternalInput").ap()
    aps["sin_st"] = nc.dram_tensor("sin_st", [128, L_], F32, kind="ExternalInput").ap()
    aps["trilneg"] = nc.dram_tensor("trilneg", [128, 128], BF16, kind="ExternalInput").ap()
    aps["r3mask"] = nc.dram_tensor("r3mask", [128, 256], BF16, kind="ExternalInput").ap()
    aps["ident_b"] = nc.dram_tensor("ident_b", [128, 128], BF16, kind="ExternalInput").ap()
    aps["ident_f"] = nc.dram_tensor("ident_f", [128, 128], F32, kind="ExternalInput").ap()
    aps["sel2"] = nc.dram_tensor("sel2", [2, 128], F32, kind="ExternalInput").ap()
    aps["partial"] = nc.dram_tensor("partial", [L_, D], F32, kind="ExternalOutput").ap()

    with tile.TileContext(nc) as tc:
        emit(nc, tc, aps, L_)
    nc.compile()
    return nc, aps


def make_in_maps(x, Wq, Wk, Wv, Wo, L_=L):
    xT = np.ascontiguousarray(x.reshape(L_, D).T).astype(np.float32)
    consts = _host_consts(L_)
    in_maps = []
    for c in range(N_CORES):
        wqT, wkT, wvT, woC = _core_weights(c, Wq, Wk, Wv, Wo)
        m = {"xt": xT, "wq": wqT, "wk": wkT, "wv": wvT, "wo": woC}
        m.update(consts)
        in_maps.append(m)
    return in_maps


_CACHE = {}


def _run(inputs, trace=False, **kw):
    x = np.asarray(inputs["x"], np.float32)
    Wq = np.asarray(inputs["Wq"], np.float32)
    Wk = np.asarray(inputs["Wk"], np.float32)
    Wv = np.asarray(inputs["Wv"], np.float32)
    Wo = np.asarray(inputs["Wo"], np.float32)
    if "nc" not in _CACHE:
        _CACHE["nc"] = build()[0]
    nc = _CACHE["nc"]
    in_maps = make_in_maps(x, Wq, Wk, Wv, Wo)
    res = run_bass_kernel_spmd(nc, in_maps, core_ids=list(range(N_CORES)),
                               trace=trace, **kw)
    acc = np.zeros((L, D), np.float64)
    for r in res.results:
        acc += r["partial"].astype(np.float64)
    out = acc.astype(np.float32).reshape(B, L, D)
    return out, res


def kernel(**inputs):
    out, _ = _run(inputs)
    return out


# revision 19
# speedup vs baseline: 1.3263x; 1.2852x over previous
"""Trainium2 Bass kernel for nn_BloqueAttn: causal RoPE attention, 16 heads,
head-sharded (tensor-parallel) across 8 NeuronCores, o_proj row-sharded with
host-side all-reduce of the partials.

Self-contained: hardcodes shapes B=1, L=4096, D=1024, H=16, DH=64, 8 cores.
"""
import os

os.environ.setdefault("BASS_NEVER_TRACE", "1")

import numpy as np
import ml_dtypes

import concourse.bass as bass
import concourse.bacc as bacc
import concourse.mybir as mybir
import concourse.tile as tile
from concourse.bass_utils import run_bass_kernel_spmd

F32 = mybir.dt.float32
F32R = mybir.dt.float32r
BF16 = mybir.dt.bfloat16

B, L, D = 1, 4096, 1024
H, DH = 16, 64
BASE = 10000.0
N_CORES = 8
HPC = H // N_CORES          # heads per core = 2
DH2 = HPC * DH              # packed head dim = 128
SCALE = DH ** -0.5          # 0.125

NEG = -1e30


# ---------------------------------------------------------------- host helpers

def _rope_tables(L_, dh):
    inv_freq = 1.0 / (BASE ** (np.arange(0, dh, 2, dtype=np.float32) / dh))
    freqs = np.outer(np.arange(L_, dtype=np.float32), inv_freq)  # [L, 32]
    return np.cos(freqs).astype(np.float32), np.sin(freqs).astype(np.float32)


def _host_consts(L_):
    cos, sin = _rope_tables(L_, DH)          # [L, 32]
    cosT, sinT = cos.T.copy(), sin.T.copy()  # [32, L]
    cos_stack = np.concatenate([cosT, cosT, cosT, cosT], 0)          # [128, L]
    sin_signed = np.concatenate([-sinT, sinT, -sinT, sinT], 0)       # [128, L]

    j = np.arange(128)[:, None]
    c = np.arange(128)[None, :]
    trilneg = np.where(j > c, NEG, 0.0).astype(np.float32)           # [128,128]
    r3mask = np.concatenate(
        [np.full((128, 128), NEG, np.float32), trilneg], axis=1)     # [128,256]

    ident = np.eye(128, dtype=np.float32)
    sel2 = np.zeros((2, 128), np.float32)
    sel2[0, 0:64] = 1.0
    sel2[1, 64:128] = 1.0
    return {
        "cos_st": cos_stack,
        "sin_st": sin_signed,
        "trilneg": trilneg.astype(ml_dtypes.bfloat16),
        "r3mask": r3mask.astype(ml_dtypes.bfloat16),
        "ident_b": ident.astype(ml_dtypes.bfloat16),
        "ident_f": ident,
        "sel2": sel2,
    }


def _core_weights(core, Wq, Wk, Wv, Wo):
    """Per-core transposed weight slices with RoPE even/odd permutation."""
    perm = np.concatenate([np.arange(0, DH, 2), np.arange(1, DH, 2)])  # [64]
    rows_p, rows = [], []
    for hh in (HPC * core, HPC * core + 1):
        rows_p.append(DH * hh + perm)
        rows.append(DH * hh + np.arange(DH))
    rows_p = np.concatenate(rows_p)
    rows = np.concatenate(rows)
    wqT = np.ascontiguousarray(Wq[rows_p, :].T)   # [D, 128]
    wkT = np.ascontiguousarray(Wk[rows_p, :].T)   # [D, 128]
    wvT = np.ascontiguousarray(Wv[rows, :].T)     # [D, 128]
    woC = np.ascontiguousarray(Wo[:, DH2 * core: DH2 * (core + 1)].T)  # [128, D]
    return wqT, wkT, wvT, woC


# ---------------------------------------------------------------- device emit

def emit(nc, tc, aps, L_):
    """Emit the per-core program. aps: dict of dram APs."""
    NLT = L_ // 512           # 512-wide L tiles
    ND = D // 128             # D chunks = 8
    NQB = NLT                 # q tiles of 512

    xt = aps["xt"]
    partial = aps["partial"]

    with tc.tile_pool(name="persist", bufs=1) as pp:
        wq_c = [pp.tile([128, 128], F32R, name=f"wq_c{i}") for i in range(ND)]
        wk_c = [pp.tile([128, 128], F32R, name=f"wk_c{i}") for i in range(ND)]
        wv_c = [pp.tile([128, 128], F32R, name=f"wv_c{i}") for i in range(ND)]
        wo_sb = pp.tile([128, D], F32R)
        cos_sb = pp.tile([128, L_], F32)
        sin_sb = pp.tile([128, L_], F32)
        tril_sb = pp.tile([128, 128], BF16)
        r3m_sb = pp.tile([128, 256], BF16)
        idb_sb = pp.tile([128, 128], BF16)
        idf_sb = pp.tile([128, 128], F32)
        sel2_sb = pp.tile([2, 128], F32)
        qT = pp.tile([128, L_], F32R)
        kT = pp.tile([128, L_], F32R)
        v_nat = pp.tile([128, (L_ // 128) * 130], F32R)
        OT = pp.tile([128, L_], F32R)
        stack16 = pp.tile([128, 512], F32)
        inv16 = pp.tile([128, 512], F32)
        inv2 = pp.tile([2, L_], F32)

        for name, dsts in (("wq", wq_c), ("wk", wk_c), ("wv", wv_c)):
            for ch in range(ND):
                nc.sync.dma_start(dsts[ch][:], aps[name][bass.ts(ch, 128), :])
        nc.sync.dma_start(wo_sb[:], aps["wo"][:])
        nc.sync.dma_start(cos_sb[:], aps["cos_st"][:])
        nc.sync.dma_start(sin_sb[:], aps["sin_st"][:])
        nc.sync.dma_start(tril_sb[:], aps["trilneg"][:])
        nc.sync.dma_start(r3m_sb[:], aps["r3mask"][:])
        nc.sync.dma_start(idb_sb[:], aps["ident_b"][:])
        nc.sync.dma_start(idf_sb[:], aps["ident_f"][:])
        nc.sync.dma_start(sel2_sb[:], aps["sel2"][:])
        nc.gpsimd.memset(v_nat[:].bitcast(F32), 1.0)

        # ---------------- Phase A: projections + RoPE + V transpose ----------
        with tc.tile_pool(name="psA", bufs=1, space="PSUM") as psA, \
             tc.tile_pool(name="sbA", bufs=1) as sbA:
            for lp in range(NLT // 2):
                sl = bass.ts(lp, 1024)
                pr = [psA.tile([128, 1024], F32, tag=t, bufs=1, name=f"pr_{t}")
                      for t in ("qps", "kps", "vps")]
                for ch in range(ND):
                    xt_t = sbA.tile([128, 1024], F32R, tag="xt", bufs=6)
                    nc.sync.dma_start(xt_t[:], xt[bass.ts(ch, 128), sl])
                    st, sp = ch == 0, ch == ND - 1
                    for wgt, ps in zip((wq_c, wk_c, wv_c), pr):
                        for half in range(2):
                            nc.tensor.matmul(ps[:, bass.ts(half, 512)], wgt[ch][:],
                                             xt_t[:, bass.ts(half, 512)],
                                             start=st, stop=sp)
                # RoPE per lp: rot = raw*cos + swap(raw)*sin_signed
                for pi, dst in ((0, qT), (1, kT)):
                    raw = sbA.tile([128, 1024], F32, tag="raw", bufs=2)
                    swp = sbA.tile([128, 1024], F32, tag="swp", bufs=2)
                    nc.scalar.copy(raw[:], pr[pi][:])
                    for b0 in range(4):
                        src_b = (b0 ^ 1) * 32
                        nc.sync.dma_start(swp[b0 * 32:(b0 + 1) * 32, :],
                                          raw[src_b:src_b + 32, :])
                    nc.vector.tensor_mul(dst[:, sl], raw[:], cos_sb[:, sl])
                    nc.vector.tensor_mul(swp[:], swp[:], sin_sb[:, sl])
                    nc.vector.tensor_add(dst[:, sl], dst[:, sl], swp[:])
                # V via PE transpose
                vt = sbA.tile([128, 1024], F32, tag="vt", bufs=2)
                nc.scalar.copy(vt[:], pr[2][:])
                for j in range(8):
                    kb = 8 * lp + j
                    trp = psA.tile([128, 128], F32, tag="trp", bufs=2)
                    nc.tensor.transpose(trp[:], vt[:, bass.ts(j, 128)], idf_sb[:])
                    nc.scalar.copy(v_nat[:, 130 * kb:130 * kb + 64], trp[:, 0:64])
                    nc.scalar.copy(v_nat[:, 130 * kb + 65:130 * kb + 129], trp[:, 64:128])

        # ---------------- Phase B: attention + pipelined norm/o_proj ---------
        def norm_and_oproj(psNC, sbC, qb_lo, qb_hi):
            base = 32 * (qb_lo // (NQB // 4)) if NQB >= 4 else 0
            nh = qb_hi - qb_lo
            for qb in range(qb_lo, qb_hi):
                r0 = base + (qb - qb_lo)
                r1 = base + nh + (qb - qb_lo)
                nc.sync.dma_start(stack16[r0:r0 + 1, :], sums_sb[qb][64:65, 0:512])
                nc.sync.dma_start(stack16[r1:r1 + 1, :], sums_sb[qb][64:65, 512:1024])
            nc.vector.reciprocal(inv16[base:base + 2 * nh, :],
                                 stack16[base:base + 2 * nh, :])
            for qb in range(qb_lo, qb_hi):
                osl = bass.ts(qb, 512)
                r0 = base + (qb - qb_lo)
                r1 = base + nh + (qb - qb_lo)
                nc.sync.dma_start(inv2[0:1, osl], inv16[r0:r0 + 1, :])
                nc.sync.dma_start(inv2[1:2, osl], inv16[r1:r1 + 1, :])
            for qb in range(qb_lo, qb_hi):
                osl = bass.ts(qb, 512)
                bc = psNC.tile([128, 512], F32, tag="op", bufs=2)
                nc.tensor.matmul(bc[:], sel2_sb[:], inv2[:, osl],
                                 start=True, stop=True)
                nc.vector.tensor_mul(OT[:, osl], OT[:, osl], bc[:])
            for lc in range(4 * qb_lo, 4 * qb_hi):
                ob = sbC.tile([128, 1024], F32, tag="ob")
                for n in range(D // 512):
                    op = psNC.tile([128, 512], F32, tag="op", bufs=2)
                    nc.tensor.matmul(op[:], OT[:, bass.ts(lc, 128)],
                                     wo_sb[:, bass.ts(n, 512)],
                                     start=True, stop=True)
                    nc.vector.tensor_copy(ob[:, bass.ts(n, 512)], op[:])
                nc.sync.dma_start(partial[bass.ts(lc, 128), :], ob[:])

        sums_sb = {}
        with tc.tile_pool(name="psB", bufs=1, space="PSUM") as psB, \
             tc.tile_pool(name="psNC", bufs=1, space="PSUM") as psNC, \
             tc.tile_pool(name="sbC", bufs=3) as sbC, \
             tc.tile_pool(name="sbB", bufs=4) as sbB:
            for qb in range(NQB):
                qsl0 = 512 * qb
                ov0 = psB.tile([128, 512], F32, tag="pv0", bufs=1)
                ov1 = psB.tile([128, 512], F32, tag="pv1", bufs=1)
                nkb = 4 * qb + 4
                for kb in range(nkb):
                    r = kb - 4 * qb
                    if r < 0:
                        c0 = 0
                    elif r < 3:
                        c0 = 128 * r
                    else:
                        c0 = 256
                    W = 512 - c0
                    ksl = bass.ts(kb, 128)
                    qsl = bass.ds(qsl0 + c0, W)
                    s01 = psB.tile([128, 1024], F32, tag="sc", bufs=2)
                    diag = r >= 0
                    nc.tensor.matmul(s01[:, c0:512], kT[0:64, ksl],
                                     qT[0:64, qsl], start=True, stop=not diag)
                    nc.tensor.matmul(s01[:, 512 + c0:1024], kT[64:128, ksl],
                                     qT[64:128, qsl], start=True, stop=not diag)
                    if diag:
                        if r < 3:
                            msl, mk = slice(128 * r, 128 * r + 128), tril_sb
                        else:
                            msl, mk = slice(256, 512), r3m_sb
                        nc.tensor.matmul(s01[:, msl], idb_sb[:], mk[:],
                                         start=False, stop=True)
                        nc.tensor.matmul(s01[:, msl.start + 512:msl.stop + 512],
                                         idb_sb[:], mk[:], start=False, stop=True)
                    p01 = sbB.tile([128, 1024], F32R, tag="p01", bufs=6)
                    sin_ = s01[:].rearrange("p (h c) -> p h c", h=2)[:, :, c0:512]
                    pout = p01[:].rearrange("p (h c) -> p h c", h=2)[:, :, c0:512]
                    nc.scalar.activation(pout, sin_,
                                         mybir.ActivationFunctionType.Exp, scale=SCALE)
                    st, sp = kb == 0, kb == nkb - 1
                    nc.tensor.matmul(ov0[0:65, c0:512],
                                     v_nat[:, bass.ds(130 * kb, 65)],
                                     p01[:, c0:512], start=st, stop=sp)
                    nc.tensor.matmul(ov1[0:65, c0:512],
                                     v_nat[:, bass.ds(130 * kb + 65, 65)],
                                     p01[:, 512 + c0:1024], start=st, stop=sp)
                osl = bass.ds(qsl0, 512)
                nc.vector.tensor_copy(OT[0:64, osl], ov0[0:64, :])
                o1t = sbB.tile([64, 512], F32R, tag="o1t", bufs=2)
                nc.vector.tensor_copy(o1t[:], ov1[0:64, :])
                nc.sync.dma_start(OT[64:128, osl], o1t[:])
                sm = sbB.tile([65, 1024], F32, tag="sums", bufs=2)
                sums_sb[qb] = sm
                nc.vector.tensor_copy(sm[64:65, 0:512], ov0[64:65, :])
                nc.vector.tensor_copy(sm[64:65, 512:1024], ov1[64:65, :])
                if NQB >= 4:
                    step = NQB // 4
                    if (qb + 1) % step == 0:
                        norm_and_oproj(psNC, sbC, qb + 1 - step, qb + 1)
                else:
                    if qb == NQB - 1:
                        norm_and_oproj(psNC, sbC, 0, NQB)


def build(L_=L, debug=False):
    nc = bacc.Bacc("TRN2", target_bir_lowering=False, debug=debug,
                   enable_asserts=False)
    aps = {}
    aps["xt"] = nc.dram_tensor("xt", [D, L_], F32R, kind="ExternalInput").ap()
    aps["wq"] = nc.dram_tensor("wq", [D, 128], F32R, kind="ExternalInput").ap()
    aps["wk"] = nc.dram_tensor("wk", [D, 128], F32R, kind="ExternalInput").ap()
    aps["wv"] = nc.dram_tensor("wv", [D, 128], F32R, kind="ExternalInput").ap()
    aps["wo"] = nc.dram_tensor("wo", [128, D], F32R, kind="ExternalInput").ap()
    aps["cos_st"] = nc.dram_tensor("cos_st", [128, L_], F32, kind="ExternalInput").ap()
    aps["sin_st"] = nc.dram_tensor("sin_st", [128, L_], F32, kind="ExternalInput").ap()
    aps["trilneg"] = nc.dram_tensor("trilneg", [128, 128], BF16, kind="ExternalInput").ap()
    aps["r3mask"] = nc.dram_tensor("r3mask", [128, 256], BF16, kind="ExternalInput").ap()
    aps["ident_b"] = nc.dram_tensor("ident_b", [128, 128], BF16, kind="ExternalInput").ap()
    aps["ident_f"] = nc.dram_tensor("ident_f", [128, 128], F32, kind="ExternalInput").ap()
    aps["sel2"] = nc.dram_tensor("sel2", [2, 128], F32, kind="ExternalInput").ap()
    aps["partial"] = nc.dram_tensor("partial", [L_, D], F32, kind="ExternalOutput").ap()

    with tile.TileContext(nc) as tc:
        emit(nc, tc, aps, L_)
    nc.compile()
    return nc, aps


def make_in_maps(x, Wq, Wk, Wv, Wo, L_=L):
    xT = np.ascontiguousarray(x.reshape(L_, D).T).astype(np.float32)
    consts = _host_consts(L_)
    in_maps = []
    for c in range(N_CORES):
        wqT, wkT, wvT, woC = _core_weights(c, Wq, Wk, Wv, Wo)
        m = {"xt": xT, "wq": wqT, "wk": wkT, "wv": wvT, "wo": woC}
        m.update(consts)
        in_maps.append(m)
    return in_maps


_CACHE = {}


def _run(inputs, trace=False, **kw):
    if trace:
        os.environ.pop("BASS_NEVER_TRACE", None)
    x = np.asarray(inputs["x"], np.float32)
    Wq = np.asarray(inputs["Wq"], np.float32)
    Wk = np.asarray(inputs["Wk"], np.float32)
    Wv = np.asarray(inputs["Wv"], np.float32)
    Wo = np.asarray(inputs["Wo"], np.float32)
    if "nc" not in _CACHE:
        _CACHE["nc"] = build()[0]
    nc = _CACHE["nc"]
    in_maps = make_in_maps(x, Wq, Wk, Wv, Wo)
    res = run_bass_kernel_spmd(nc, in_maps, core_ids=list(range(N_CORES)),
                               trace=trace, **kw)
    acc = np.zeros((L, D), np.float64)
    for r in res.results:
        acc += r["partial"].astype(np.float64)
    out = acc.astype(np.float32).reshape(B, L, D)
    return out, res


def kernel(**inputs):
    out, _ = _run(inputs)
    return out


# revision 22
# speedup vs baseline: 1.3808x; 1.0411x over previous
"""Trainium2 Bass kernel for nn_BloqueAttn: causal RoPE attention, 16 heads,
head-sharded (tensor-parallel) across 8 NeuronCores, o_proj row-sharded with
host-side all-reduce of the partials.

Self-contained: hardcodes shapes B=1, L=4096, D=1024, H=16, DH=64, 8 cores.
"""
import os

os.environ.setdefault("BASS_NEVER_TRACE", "1")

import numpy as np
import ml_dtypes

import concourse.bass as bass
import concourse.bacc as bacc
import concourse.mybir as mybir
import concourse.tile as tile
from concourse.bass_utils import run_bass_kernel_spmd

F32 = mybir.dt.float32
F32R = mybir.dt.float32r
BF16 = mybir.dt.bfloat16

B, L, D = 1, 4096, 1024
H, DH = 16, 64
BASE = 10000.0
N_CORES = 8
HPC = H // N_CORES          # heads per core = 2
DH2 = HPC * DH              # packed head dim = 128
SCALE = DH ** -0.5          # 0.125

NEG = -1e30


# ---------------------------------------------------------------- host helpers

def _rope_tables(L_, dh):
    inv_freq = 1.0 / (BASE ** (np.arange(0, dh, 2, dtype=np.float32) / dh))
    freqs = np.outer(np.arange(L_, dtype=np.float32), inv_freq)  # [L, 32]
    return np.cos(freqs).astype(np.float32), np.sin(freqs).astype(np.float32)


def _host_consts(L_):
    cos, sin = _rope_tables(L_, DH)          # [L, 32]
    cosT, sinT = cos.T.copy(), sin.T.copy()  # [32, L]
    cos_stack = np.concatenate([cosT, cosT, cosT, cosT], 0)          # [128, L]
    sin_signed = np.concatenate([-sinT, sinT, -sinT, sinT], 0)       # [128, L]

    j = np.arange(128)[:, None]
    c = np.arange(128)[None, :]
    trilneg = np.where(j > c, NEG, 0.0).astype(np.float32)           # [128,128]
    r3mask = np.concatenate(
        [np.full((128, 128), NEG, np.float32), trilneg], axis=1)     # [128,256]

    ident = np.eye(128, dtype=np.float32)
    sel2 = np.zeros((2, 128), np.float32)
    sel2[0, 0:64] = 1.0
    sel2[1, 64:128] = 1.0
    return {
        "cos_st": cos_stack,
        "sin_st": sin_signed,
        "trilneg": trilneg.astype(ml_dtypes.bfloat16),
        "r3mask": r3mask.astype(ml_dtypes.bfloat16),
        "ident_b": ident.astype(ml_dtypes.bfloat16),
        "ident_f": ident,
        "sel2": sel2,
    }


def _core_weights(core, Wq, Wk, Wv, Wo):
    """Per-core transposed weight slices with RoPE even/odd permutation."""
    perm = np.concatenate([np.arange(0, DH, 2), np.arange(1, DH, 2)])  # [64]
    rows_p, rows = [], []
    for hh in (HPC * core, HPC * core + 1):
        rows_p.append(DH * hh + perm)
        rows.append(DH * hh + np.arange(DH))
    rows_p = np.concatenate(rows_p)
    rows = np.concatenate(rows)
    wqT = np.ascontiguousarray(Wq[rows_p, :].T)   # [D, 128]
    wkT = np.ascontiguousarray(Wk[rows_p, :].T)   # [D, 128]
    wvT = np.ascontiguousarray(Wv[rows, :].T)     # [D, 128]
    woC = np.ascontiguousarray(Wo[:, DH2 * core: DH2 * (core + 1)].T)  # [128, D]
    return wqT, wkT, wvT, woC


# ---------------------------------------------------------------- device emit

def emit(nc, tc, aps, L_):
    """Emit the per-core program. aps: dict of dram APs."""
    NLT = L_ // 512           # 512-wide L tiles
    ND = D // 128             # D chunks = 8
    NQB = NLT                 # q tiles of 512

    xt = aps["xt"]
    partial = aps["partial"]

    with tc.tile_pool(name="persist", bufs=1) as pp:
        wq_c = [pp.tile([128, 128], F32R, name=f"wq_c{i}") for i in range(ND)]
        wk_c = [pp.tile([128, 128], F32R, name=f"wk_c{i}") for i in range(ND)]
        wv_c = [pp.tile([128, 128], F32R, name=f"wv_c{i}") for i in range(ND)]
        wo_sb = pp.tile([128, D], F32R)
        cos_sb = pp.tile([128, L_], F32)
        sin_sb = pp.tile([128, L_], F32)
        tril_sb = pp.tile([128, 128], BF16)
        r3m_sb = pp.tile([128, 256], BF16)
        idb_sb = pp.tile([128, 128], BF16)
        idf_sb = pp.tile([128, 128], F32)
        sel2_sb = pp.tile([2, 128], F32R)
        qT = pp.tile([128, L_], F32R)
        kT = pp.tile([128, L_], F32R)
        v_nat = pp.tile([128, (L_ // 128) * 130], F32R)
        OT = pp.tile([128, L_], F32R)
        stack16 = pp.tile([128, 512], F32)
        inv16 = pp.tile([128, 512], F32R)
        inv2 = pp.tile([2, L_], F32R)

        for name, dsts in (("wq", wq_c), ("wk", wk_c), ("wv", wv_c)):
            for ch in range(ND):
                nc.sync.dma_start(dsts[ch][:], aps[name][bass.ts(ch, 128), :])
        nc.sync.dma_start(wo_sb[:], aps["wo"][:])
        nc.sync.dma_start(cos_sb[:], aps["cos_st"][:])
        nc.sync.dma_start(sin_sb[:], aps["sin_st"][:])
        nc.sync.dma_start(tril_sb[:], aps["trilneg"][:])
        nc.sync.dma_start(r3m_sb[:], aps["r3mask"][:])
        nc.sync.dma_start(idb_sb[:], aps["ident_b"][:])
        nc.sync.dma_start(idf_sb[:], aps["ident_f"][:])
        nc.sync.dma_start(sel2_sb[:], aps["sel2"][:])
        nc.gpsimd.memset(v_nat[:].bitcast(F32), 1.0)

        # ---------------- Phase A: projections + RoPE + V transpose ----------
        with tc.tile_pool(name="psA", bufs=1, space="PSUM") as psA, \
             tc.tile_pool(name="sbA", bufs=1) as sbA:
            for lp in range(NLT // 2):
                sl = bass.ts(lp, 1024)
                pr = [psA.tile([128, 1024], F32, tag=t, bufs=1, name=f"pr_{t}")
                      for t in ("qps", "kps", "vps")]
                for ch in range(ND):
                    xt_t = sbA.tile([128, 1024], F32R, tag="xt", bufs=6)
                    nc.sync.dma_start(xt_t[:], xt[bass.ts(ch, 128), sl])
                    st, sp = ch == 0, ch == ND - 1
                    for wgt, ps in zip((wq_c, wk_c, wv_c), pr):
                        for half in range(2):
                            nc.tensor.matmul(ps[:, bass.ts(half, 512)], wgt[ch][:],
                                             xt_t[:, bass.ts(half, 512)],
                                             start=st, stop=sp)
                # RoPE per lp: rot = raw*cos + swap(raw)*sin_signed
                for pi, dst in ((0, qT), (1, kT)):
                    raw = sbA.tile([128, 1024], F32, tag="raw", bufs=2)
                    swp = sbA.tile([128, 1024], F32, tag="swp", bufs=2)
                    nc.scalar.copy(raw[:], pr[pi][:])
                    for b0 in range(4):
                        src_b = (b0 ^ 1) * 32
                        nc.sync.dma_start(swp[b0 * 32:(b0 + 1) * 32, :],
                                          raw[src_b:src_b + 32, :])
                    nc.vector.tensor_mul(dst[:, sl], raw[:], cos_sb[:, sl])
                    nc.vector.tensor_mul(swp[:], swp[:], sin_sb[:, sl])
                    nc.vector.tensor_add(dst[:, sl], dst[:, sl], swp[:])
                # V via PE transpose
                vt = sbA.tile([128, 1024], F32, tag="vt", bufs=2)
                nc.scalar.copy(vt[:], pr[2][:])
                for j in range(8):
                    kb = 8 * lp + j
                    trp = psA.tile([128, 128], F32, tag="trp", bufs=2)
                    nc.tensor.transpose(trp[:], vt[:, bass.ts(j, 128)], idf_sb[:])
                    nc.scalar.copy(v_nat[:, 130 * kb:130 * kb + 64], trp[:, 0:64])
                    nc.scalar.copy(v_nat[:, 130 * kb + 65:130 * kb + 129], trp[:, 64:128])

        # ---------------- Phase B: attention + pipelined norm/o_proj ---------
        def norm_and_oproj(psNC, sbC, qb_lo, qb_hi, base=None):
            if base is None:
                base = 32 * (qb_lo // (NQB // 4)) if NQB >= 4 else 0
            nh = qb_hi - qb_lo
            for qb in range(qb_lo, qb_hi):
                r0 = base + (qb - qb_lo)
                r1 = base + nh + (qb - qb_lo)
                nc.sync.dma_start(stack16[r0:r0 + 1, :], sums_sb[qb][64:65, 0:512])
                nc.sync.dma_start(stack16[r1:r1 + 1, :], sums_sb[qb][64:65, 512:1024])
            with nc.allow_low_precision(reason="f32r reciprocal for bcast matmul"):
                nc.vector.reciprocal(inv16[base:base + 2 * nh, :],
                                     stack16[base:base + 2 * nh, :])
            for qb in range(qb_lo, qb_hi):
                osl = bass.ts(qb, 512)
                r0 = base + (qb - qb_lo)
                r1 = base + nh + (qb - qb_lo)
                nc.sync.dma_start(inv2[0:1, osl], inv16[r0:r0 + 1, :])
                nc.sync.dma_start(inv2[1:2, osl], inv16[r1:r1 + 1, :])
            for qb in range(qb_lo, qb_hi):
                osl = bass.ts(qb, 512)
                bc = psNC.tile([128, 512], F32, tag="op", bufs=2)
                nc.tensor.matmul(bc[:], sel2_sb[:], inv2[:, osl],
                                 start=True, stop=True)
                nc.vector.tensor_mul(OT[:, osl], OT[:, osl], bc[:])
            for lc in range(4 * qb_lo, 4 * qb_hi):
                ob = sbC.tile([128, 1024], F32, tag="ob")
                for n in range(D // 512):
                    op = psNC.tile([128, 512], F32, tag="op", bufs=2)
                    nc.tensor.matmul(op[:], OT[:, bass.ts(lc, 128)],
                                     wo_sb[:, bass.ts(n, 512)],
                                     start=True, stop=True)
                    nc.vector.tensor_copy(ob[:, bass.ts(n, 512)], op[:])
                nc.sync.dma_start(partial[bass.ts(lc, 128), :], ob[:])

        sums_sb = {}
        with tc.tile_pool(name="psB", bufs=1, space="PSUM") as psB, \
             tc.tile_pool(name="psNC", bufs=1, space="PSUM") as psNC, \
             tc.tile_pool(name="sbC", bufs=3) as sbC, \
             tc.tile_pool(name="sbB", bufs=4) as sbB:
            for qb in range(NQB):
                qsl0 = 512 * qb
                ov0 = psB.tile([128, 512], F32, tag="pv0", bufs=1)
                ov1 = psB.tile([128, 512], F32, tag="pv1", bufs=1)
                nkb = 4 * qb + 4
                for kb in range(nkb):
                    r = kb - 4 * qb
                    if r < 0:
                        c0 = 0
                    elif r < 3:
                        c0 = 128 * r
                    else:
                        c0 = 256
                    W = 512 - c0
                    ksl = bass.ts(kb, 128)
                    qsl = bass.ds(qsl0 + c0, W)
                    s01 = psB.tile([128, 1024], F32, tag="sc", bufs=2)
                    diag = r >= 0
                    nc.tensor.matmul(s01[:, c0:512], kT[0:64, ksl],
                                     qT[0:64, qsl], start=True, stop=not diag)
                    nc.tensor.matmul(s01[:, 512 + c0:1024], kT[64:128, ksl],
                                     qT[64:128, qsl], start=True, stop=not diag)
                    if diag:
                        if r < 3:
                            msl, mk = slice(128 * r, 128 * r + 128), tril_sb
                        else:
                            msl, mk = slice(256, 512), r3m_sb
                        nc.tensor.matmul(s01[:, msl], idb_sb[:], mk[:],
                                         start=False, stop=True)
                        nc.tensor.matmul(s01[:, msl.start + 512:msl.stop + 512],
                                         idb_sb[:], mk[:], start=False, stop=True)
                    p01 = sbB.tile([128, 1024], F32R, tag="p01", bufs=6)
                    sin_ = s01[:].rearrange("p (h c) -> p h c", h=2)[:, :, c0:512]
                    pout = p01[:].rearrange("p (h c) -> p h c", h=2)[:, :, c0:512]
                    nc.scalar.activation(pout, sin_,
                                         mybir.ActivationFunctionType.Exp, scale=SCALE)
                    st, sp = kb == 0, kb == nkb - 1
                    nc.tensor.matmul(ov0[0:65, c0:512],
                                     v_nat[:, bass.ds(130 * kb, 65)],
                                     p01[:, c0:512], start=st, stop=sp)
                    nc.tensor.matmul(ov1[0:65, c0:512],
                                     v_nat[:, bass.ds(130 * kb + 65, 65)],
                                     p01[:, 512 + c0:1024], start=st, stop=sp)
                osl = bass.ds(qsl0, 512)
                nc.vector.tensor_copy(OT[0:64, osl], ov0[0:64, :])
                o1t = sbB.tile([64, 512], F32R, tag="o1t", bufs=2)
                nc.vector.tensor_copy(o1t[:], ov1[0:64, :])
                nc.sync.dma_start(OT[64:128, osl], o1t[:])
                sm = sbB.tile([65, 1024], F32, tag="sums", bufs=2)
                sums_sb[qb] = sm
                nc.vector.tensor_copy(sm[64:65, 0:512], ov0[64:65, :])
                nc.vector.tensor_copy(sm[64:65, 512:1024], ov1[64:65, :])
                if NQB >= 4:
                    step = NQB // 4
                    if qb >= NQB - step:
                        # last quarter: per-qb for a shorter serial tail
                        norm_and_oproj(psNC, sbC, qb, qb + 1,
                                       base=32 * (qb - (NQB - step)))
                    elif (qb + 1) % step == 0:
                        norm_and_oproj(psNC, sbC, qb + 1 - step, qb + 1)
                else:
                    if qb == NQB - 1:
                        norm_and_oproj(psNC, sbC, 0, NQB)


def build(L_=L, debug=False):
    nc = bacc.Bacc("TRN2", target_bir_lowering=False, debug=debug,
                   enable_asserts=False)
    aps = {}
    aps["xt"] = nc.dram_tensor("xt", [D, L_], F32R, kind="ExternalInput").ap()
    aps["wq"] = nc.dram_tensor("wq", [D, 128], F32R, kind="ExternalInput").ap()
    aps["wk"] = nc.dram_tensor("wk", [D, 128], F32R, kind="ExternalInput").ap()
    aps["wv"] = nc.dram_tensor("wv", [D, 128], F32R, kind="ExternalInput").ap()
    aps["wo"] = nc.dram_tensor("wo", [128, D], F32R, kind="ExternalInput").ap()
    aps["cos_st"] = nc.dram_tensor("cos_st", [128, L_], F32, kind="ExternalInput").ap()
    aps["sin_st"] = nc.dram_tensor("sin_st", [128, L_], F32, kind="ExternalInput").ap()
    aps["trilneg"] = nc.dram_tensor("trilneg", [128, 128], BF16, kind="ExternalInput").ap()
    aps["r3mask"] = nc.dram_tensor("r3mask", [128, 256], BF16, kind="ExternalInput").ap()
    aps["ident_b"] = nc.dram_tensor("ident_b", [128, 128], BF16, kind="ExternalInput").ap()
    aps["ident_f"] = nc.dram_tensor("ident_f", [128, 128], F32, kind="ExternalInput").ap()
    aps["sel2"] = nc.dram_tensor("sel2", [2, 128], F32R, kind="ExternalInput").ap()
    aps["partial"] = nc.dram_tensor("partial", [L_, D], F32, kind="ExternalOutput").ap()

    with tile.TileContext(nc) as tc:
        emit(nc, tc, aps, L_)
    nc.compile()
    return nc, aps


def make_in_maps(x, Wq, Wk, Wv, Wo, L_=L):
    xT = np.ascontiguousarray(x.reshape(L_, D).T).astype(np.float32)
    consts = _host_consts(L_)
    in_maps = []
    for c in range(N_CORES):
        wqT, wkT, wvT, woC = _core_weights(c, Wq, Wk, Wv, Wo)
        m = {"xt": xT, "wq": wqT, "wk": wkT, "wv": wvT, "wo": woC}
        m.update(consts)
        in_maps.append(m)
    return in_maps


_CACHE = {}


def _run(inputs, trace=False, **kw):
    if trace:
        os.environ.pop("BASS_NEVER_TRACE", None)
    x = np.asarray(inputs["x"], np.float32)
    Wq = np.asarray(inputs["Wq"], np.float32)
    Wk = np.asarray(inputs["Wk"], np.float32)
    Wv = np.asarray(inputs["Wv"], np.float32)
    Wo = np.asarray(inputs["Wo"], np.float32)
    if "nc" not in _CACHE:
        _CACHE["nc"] = build()[0]
    nc = _CACHE["nc"]
    in_maps = make_in_maps(x, Wq, Wk, Wv, Wo)
    res = run_bass_kernel_spmd(nc, in_maps, core_ids=list(range(N_CORES)),
                               trace=trace, **kw)
    acc = np.zeros((L, D), np.float64)
    for r in res.results:
        acc += r["partial"].astype(np.float64)
    out = acc.astype(np.float32).reshape(B, L, D)
    return out, res


def kernel(**inputs):
    out, _ = _run(inputs)
    return out


# revision 25
# speedup vs baseline: 1.3880x; 1.0052x over previous
"""Trainium2 Bass kernel for nn_BloqueAttn: causal RoPE attention, 16 heads,
head-sharded (tensor-parallel) across 8 NeuronCores, o_proj row-sharded with
host-side all-reduce of the partials.

Self-contained: hardcodes shapes B=1, L=4096, D=1024, H=16, DH=64, 8 cores.
"""
import os

os.environ.setdefault("BASS_NEVER_TRACE", "1")

import numpy as np
import ml_dtypes

import concourse.bass as bass
import concourse.bacc as bacc
import concourse.mybir as mybir
import concourse.tile as tile
from concourse.bass_utils import run_bass_kernel_spmd

F32 = mybir.dt.float32
F32R = mybir.dt.float32r
BF16 = mybir.dt.bfloat16

B, L, D = 1, 4096, 1024
H, DH = 16, 64
BASE = 10000.0
N_CORES = 8
HPC = H // N_CORES          # heads per core = 2
DH2 = HPC * DH              # packed head dim = 128
SCALE = DH ** -0.5          # 0.125

NEG = -1e30


# ---------------------------------------------------------------- host helpers

def _rope_tables(L_, dh):
    inv_freq = 1.0 / (BASE ** (np.arange(0, dh, 2, dtype=np.float32) / dh))
    freqs = np.outer(np.arange(L_, dtype=np.float32), inv_freq)  # [L, 32]
    return np.cos(freqs).astype(np.float32), np.sin(freqs).astype(np.float32)


def _host_consts(L_):
    cos, sin = _rope_tables(L_, DH)          # [L, 32]
    cosT, sinT = cos.T.copy(), sin.T.copy()  # [32, L]
    cos_stack = np.concatenate([cosT, cosT, cosT, cosT], 0)          # [128, L]
    sin_signed = np.concatenate([-sinT, sinT, -sinT, sinT], 0)       # [128, L]

    j = np.arange(128)[:, None]
    c = np.arange(128)[None, :]
    trilneg = np.where(j > c, NEG, 0.0).astype(np.float32)           # [128,128]
    r3mask = np.concatenate(
        [np.full((128, 128), NEG, np.float32), trilneg], axis=1)     # [128,256]

    ident = np.eye(128, dtype=np.float32)
    sel2 = np.zeros((2, 128), np.float32)
    sel2[0, 0:64] = 1.0
    sel2[1, 64:128] = 1.0
    return {
        "cos_st": cos_stack,
        "sin_st": sin_signed,
        "trilneg": trilneg.astype(ml_dtypes.bfloat16),
        "r3mask": r3mask.astype(ml_dtypes.bfloat16),
        "ident_b": ident.astype(ml_dtypes.bfloat16),
        "ident_f": ident,
        "sel2": sel2,
    }


def _core_weights(core, Wq, Wk, Wv, Wo):
    """Per-core transposed weight slices with RoPE even/odd permutation."""
    perm = np.concatenate([np.arange(0, DH, 2), np.arange(1, DH, 2)])  # [64]
    rows_p, rows = [], []
    for hh in (HPC * core, HPC * core + 1):
        rows_p.append(DH * hh + perm)
        rows.append(DH * hh + np.arange(DH))
    rows_p = np.concatenate(rows_p)
    rows = np.concatenate(rows)
    wqT = np.ascontiguousarray(Wq[rows_p, :].T)   # [D, 128]
    wkT = np.ascontiguousarray(Wk[rows_p, :].T)   # [D, 128]
    wvT = np.ascontiguousarray(Wv[rows, :].T)     # [D, 128]
    woC = np.ascontiguousarray(Wo[:, DH2 * core: DH2 * (core + 1)].T)  # [128, D]
    return wqT, wkT, wvT, woC


# ---------------------------------------------------------------- device emit

def emit(nc, tc, aps, L_):
    """Emit the per-core program. aps: dict of dram APs."""
    NLT = L_ // 512           # 512-wide L tiles
    ND = D // 128             # D chunks = 8
    NQB = NLT                 # q tiles of 512

    xt = aps["xt"]
    partial = aps["partial"]

    with tc.tile_pool(name="persist", bufs=1) as pp:
        wq_c = [pp.tile([128, 128], F32R, name=f"wq_c{i}") for i in range(ND)]
        wk_c = [pp.tile([128, 128], F32R, name=f"wk_c{i}") for i in range(ND)]
        wv_c = [pp.tile([128, 128], F32R, name=f"wv_c{i}") for i in range(ND)]
        wo_sb = pp.tile([128, D], F32R)
        cos_sb = pp.tile([128, L_], F32)
        sin_sb = pp.tile([128, L_], F32)
        tril_sb = pp.tile([128, 128], BF16)
        r3m_sb = pp.tile([128, 256], BF16)
        idb_sb = pp.tile([128, 128], BF16)
        idf_sb = pp.tile([128, 128], F32)
        sel2_sb = pp.tile([2, 128], F32R)
        qT = pp.tile([128, L_], F32R)
        kT = pp.tile([128, L_], F32R)
        v_nat = pp.tile([128, (L_ // 128) * 130], F32R)
        OT = pp.tile([128, L_], F32R)
        stack16 = pp.tile([128, 512], F32)
        inv16 = pp.tile([128, 512], F32R)
        inv2 = pp.tile([2, L_], F32R)

        for name, dsts in (("wq", wq_c), ("wk", wk_c), ("wv", wv_c)):
            for ch in range(ND):
                nc.sync.dma_start(dsts[ch][:], aps[name][bass.ts(ch, 128), :])
        nc.sync.dma_start(cos_sb[:], aps["cos_st"][:])
        nc.sync.dma_start(sin_sb[:], aps["sin_st"][:])
        nc.sync.dma_start(wo_sb[:], aps["wo"][:])
        nc.sync.dma_start(tril_sb[:], aps["trilneg"][:])
        nc.sync.dma_start(r3m_sb[:], aps["r3mask"][:])
        nc.sync.dma_start(idb_sb[:], aps["ident_b"][:])
        nc.sync.dma_start(idf_sb[:], aps["ident_f"][:])
        nc.sync.dma_start(sel2_sb[:], aps["sel2"][:])
        nc.gpsimd.memset(v_nat[:].bitcast(F32), 1.0)

        # ---------------- Phase A: projections + RoPE + V transpose ----------
        with tc.tile_pool(name="psA", bufs=1, space="PSUM") as psA, \
             tc.tile_pool(name="sbA", bufs=1) as sbA:
            for lp in range(NLT // 2):
                sl = bass.ts(lp, 1024)
                pr = [psA.tile([128, 1024], F32, tag=t, bufs=1, name=f"pr_{t}")
                      for t in ("qps", "kps", "vps")]
                for ch in range(ND):
                    xt_t = sbA.tile([128, 1024], F32R, tag="xt", bufs=8)
                    nc.sync.dma_start(xt_t[:], xt[bass.ts(ch, 128), sl])
                    st, sp = ch == 0, ch == ND - 1
                    for wgt, ps in zip((wq_c, wk_c, wv_c), pr):
                        for half in range(2):
                            nc.tensor.matmul(ps[:, bass.ts(half, 512)], wgt[ch][:],
                                             xt_t[:, bass.ts(half, 512)],
                                             start=st, stop=sp)
                # RoPE per lp: rot = raw*cos + swap(raw)*sin_signed
                for pi, dst in ((0, qT), (1, kT)):
                    raw = sbA.tile([128, 1024], F32, tag="raw", bufs=2)
                    swp = sbA.tile([128, 1024], F32, tag="swp", bufs=2)
                    nc.scalar.copy(raw[:], pr[pi][:])
                    for b0 in range(4):
                        src_b = (b0 ^ 1) * 32
                        nc.sync.dma_start(swp[b0 * 32:(b0 + 1) * 32, :],
                                          raw[src_b:src_b + 32, :])
                    nc.vector.tensor_mul(dst[:, sl], raw[:], cos_sb[:, sl])
                    nc.vector.tensor_mul(swp[:], swp[:], sin_sb[:, sl])
                    nc.vector.tensor_add(dst[:, sl], dst[:, sl], swp[:])
                # V via PE transpose
                vt = sbA.tile([128, 1024], F32, tag="vt", bufs=2)
                nc.scalar.copy(vt[:], pr[2][:])
                for j in range(8):
                    kb = 8 * lp + j
                    trp = psA.tile([128, 128], F32, tag="trp", bufs=2)
                    nc.tensor.transpose(trp[:], vt[:, bass.ts(j, 128)], idf_sb[:])
                    nc.scalar.copy(v_nat[:, 130 * kb:130 * kb + 64], trp[:, 0:64])
                    nc.scalar.copy(v_nat[:, 130 * kb + 65:130 * kb + 129], trp[:, 64:128])

        # ---------------- Phase B: attention + pipelined norm/o_proj ---------
        def norm_and_oproj(psNC, sbC, qb_lo, qb_hi, base=None):
            if base is None:
                base = 32 * (qb_lo // (NQB // 4)) if NQB >= 4 else 0
            nh = qb_hi - qb_lo
            for qb in range(qb_lo, qb_hi):
                r0 = base + (qb - qb_lo)
                r1 = base + nh + (qb - qb_lo)
                nc.sync.dma_start(stack16[r0:r0 + 1, :], sums_sb[qb][64:65, 0:512])
                nc.sync.dma_start(stack16[r1:r1 + 1, :], sums_sb[qb][64:65, 512:1024])
            with nc.allow_low_precision(reason="f32r reciprocal for bcast matmul"):
                nc.vector.reciprocal(inv16[base:base + 2 * nh, :],
                                     stack16[base:base + 2 * nh, :])
            for qb in range(qb_lo, qb_hi):
                osl = bass.ts(qb, 512)
                r0 = base + (qb - qb_lo)
                r1 = base + nh + (qb - qb_lo)
                nc.sync.dma_start(inv2[0:1, osl], inv16[r0:r0 + 1, :])
                nc.sync.dma_start(inv2[1:2, osl], inv16[r1:r1 + 1, :])
            for qb in range(qb_lo, qb_hi):
                osl = bass.ts(qb, 512)
                bc = psNC.tile([128, 512], F32, tag="op", bufs=2)
                nc.tensor.matmul(bc[:], sel2_sb[:], inv2[:, osl],
                                 start=True, stop=True)
                nc.vector.tensor_mul(OT[:, osl], OT[:, osl], bc[:])
            for lc in range(4 * qb_lo, 4 * qb_hi):
                ob = sbC.tile([128, 1024], F32, tag="ob")
                for n in range(D // 512):
                    op = psNC.tile([128, 512], F32, tag="op", bufs=2)
                    nc.tensor.matmul(op[:], OT[:, bass.ts(lc, 128)],
                                     wo_sb[:, bass.ts(n, 512)],
                                     start=True, stop=True)
                    nc.vector.tensor_copy(ob[:, bass.ts(n, 512)], op[:])
                nc.sync.dma_start(partial[bass.ts(lc, 128), :], ob[:])

        sums_sb = {}
        with tc.tile_pool(name="psB", bufs=1, space="PSUM") as psB, \
             tc.tile_pool(name="psNC", bufs=1, space="PSUM") as psNC, \
             tc.tile_pool(name="sbC", bufs=3) as sbC, \
             tc.tile_pool(name="sbB", bufs=4) as sbB:
            for qb in range(NQB):
                qsl0 = 512 * qb
                ov0 = psB.tile([128, 512], F32, tag="pv0", bufs=1)
                ov1 = psB.tile([128, 512], F32, tag="pv1", bufs=1)
                nkb = 4 * qb + 4
                for kb in range(nkb):
                    r = kb - 4 * qb
                    if r < 0:
                        c0 = 0
                    elif r < 3:
                        c0 = 128 * r
                    else:
                        c0 = 256
                    W = 512 - c0
                    ksl = bass.ts(kb, 128)
                    qsl = bass.ds(qsl0 + c0, W)
                    s01 = psB.tile([128, 1024], F32, tag="sc", bufs=2)
                    diag = r >= 0
                    nc.tensor.matmul(s01[:, c0:512], kT[0:64, ksl],
                                     qT[0:64, qsl], start=True, stop=not diag)
                    nc.tensor.matmul(s01[:, 512 + c0:1024], kT[64:128, ksl],
                                     qT[64:128, qsl], start=True, stop=not diag)
                    if diag:
                        if r < 3:
                            msl, mk = slice(128 * r, 128 * r + 128), tril_sb
                        else:
                            msl, mk = slice(256, 512), r3m_sb
                        nc.tensor.matmul(s01[:, msl], idb_sb[:], mk[:],
                                         start=False, stop=True)
                        nc.tensor.matmul(s01[:, msl.start + 512:msl.stop + 512],
                                         idb_sb[:], mk[:], start=False, stop=True)
                    p01 = sbB.tile([128, 1024], F32R, tag="p01", bufs=8)
                    sin_ = s01[:].rearrange("p (h c) -> p h c", h=2)[:, :, c0:512]
                    pout = p01[:].rearrange("p (h c) -> p h c", h=2)[:, :, c0:512]
                    nc.scalar.activation(pout, sin_,
                                         mybir.ActivationFunctionType.Exp, scale=SCALE)
                    st, sp = kb == 0, kb == nkb - 1
                    nc.tensor.matmul(ov0[0:65, c0:512],
                                     v_nat[:, bass.ds(130 * kb, 65)],
                                     p01[:, c0:512], start=st, stop=sp)
                    nc.tensor.matmul(ov1[0:65, c0:512],
                                     v_nat[:, bass.ds(130 * kb + 65, 65)],
                                     p01[:, 512 + c0:1024], start=st, stop=sp)
                osl = bass.ds(qsl0, 512)
                nc.vector.tensor_copy(OT[0:64, osl], ov0[0:64, :])
                o1t = sbB.tile([64, 512], F32R, tag="o1t", bufs=2)
                nc.vector.tensor_copy(o1t[:], ov1[0:64, :])
                nc.sync.dma_start(OT[64:128, osl], o1t[:])
                sm = sbB.tile([65, 1024], F32, tag="sums", bufs=2)
                sums_sb[qb] = sm
                nc.vector.tensor_copy(sm[64:65, 0:512], ov0[64:65, :])
                nc.vector.tensor_copy(sm[64:65, 512:1024], ov1[64:65, :])
                if NQB >= 4:
                    step = NQB // 4
                    if qb >= NQB - step:
                        # last quarter: per-qb for a shorter serial tail
                        norm_and_oproj(psNC, sbC, qb, qb + 1,
                                       base=32 * (qb - (NQB - step)))
                    elif (qb + 1) % step == 0:
                        norm_and_oproj(psNC, sbC, qb + 1 - step, qb + 1)
                else:
                    if qb == NQB - 1:
                        norm_and_oproj(psNC, sbC, 0, NQB)


def build(L_=L, debug=False):
    nc = bacc.Bacc("TRN2", target_bir_lowering=False, debug=debug,
                   enable_asserts=False)
    aps = {}
    aps["xt"] = nc.dram_tensor("xt", [D, L_], F32R, kind="ExternalInput").ap()
    aps["wq"] = nc.dram_tensor("wq", [D, 128], F32R, kind="ExternalInput").ap()
    aps["wk"] = nc.dram_tensor("wk", [D, 128], F32R, kind="ExternalInput").ap()
    aps["wv"] = nc.dram_tensor("wv", [D, 128], F32R, kind="ExternalInput").ap()
    aps["wo"] = nc.dram_tensor("wo", [128, D], F32R, kind="ExternalInput").ap()
    aps["cos_st"] = nc.dram_tensor("cos_st", [128, L_], F32, kind="ExternalInput").ap()
    aps["sin_st"] = nc.dram_tensor("sin_st", [128, L_], F32, kind="ExternalInput").ap()
    aps["trilneg"] = nc.dram_tensor("trilneg", [128, 128], BF16, kind="ExternalInput").ap()
    aps["r3mask"] = nc.dram_tensor("r3mask", [128, 256], BF16, kind="ExternalInput").ap()
    aps["ident_b"] = nc.dram_tensor("ident_b", [128, 128], BF16, kind="ExternalInput").ap()
    aps["ident_f"] = nc.dram_tensor("ident_f", [128, 128], F32, kind="ExternalInput").ap()
    aps["sel2"] = nc.dram_tensor("sel2", [2, 128], F32R, kind="ExternalInput").ap()
    aps["partial"] = nc.dram_tensor("partial", [L_, D], F32, kind="ExternalOutput").ap()

    with tile.TileContext(nc) as tc:
        emit(nc, tc, aps, L_)
    nc.compile()
    return nc, aps


def make_in_maps(x, Wq, Wk, Wv, Wo, L_=L):
    xT = np.ascontiguousarray(x.reshape(L_, D).T).astype(np.float32)
    consts = _host_consts(L_)
    in_maps = []
    for c in range(N_CORES):
        wqT, wkT, wvT, woC = _core_weights(c, Wq, Wk, Wv, Wo)
        m = {"xt": xT, "wq": wqT, "wk": wkT, "wv": wvT, "wo": woC}
        m.update(consts)
        in_maps.append(m)
    return in_maps


_CACHE = {}


def _run(inputs, trace=False, **kw):
    if trace:
        os.environ.pop("BASS_NEVER_TRACE", None)
    x = np.asarray(inputs["x"], np.float32)
    Wq = np.asarray(inputs["Wq"], np.float32)
    Wk = np.asarray(inputs["Wk"], np.float32)
    Wv = np.asarray(inputs["Wv"], np.float32)
    Wo = np.asarray(inputs["Wo"], np.float32)
    if "nc" not in _CACHE:
        _CACHE["nc"] = build()[0]
    nc = _CACHE["nc"]
    in_maps = make_in_maps(x, Wq, Wk, Wv, Wo)
    res = run_bass_kernel_spmd(nc, in_maps, core_ids=list(range(N_CORES)),
                               trace=trace, **kw)
    acc = np.zeros((L, D), np.float64)
    for r in res.results:
        acc += r["partial"].astype(np.float64)
    out = acc.astype(np.float32).reshape(B, L, D)
    return out, res


def kernel(**inputs):
    out, _ = _run(inputs)
    return out
